# revision 6
# baseline (speedup 1.0000x reference)
# MoE top-2 routing kernel for Trainium2, 8 NeuronCores, data-parallel over batch.
# Dense-expert formulation: every expert matmul is computed for every token tile
# (E=8 is small), combined with masked top-2 softmax weights, then projected.
# No dispatch/scatter/gather and no DRAM scratch round trips.
# Self-contained: hardcodes shapes B=8, S=2048, D=1024, E=8, TOP_K=2.
import numpy as np

B, S, D, E = 8, 2048, 1024, 8
TOPK = 2
P = 128
NKT = D // P             # 8 contraction tiles
NT = S // P              # 16 token tiles per core


def build_kernel(s_local=S):
    import concourse.bacc as bacc
    import concourse.tile as tile
    import concourse.mybir as mybir
    from concourse.masks import make_identity

    dt = mybir.dt
    nt = s_local // P

    nc = bacc.Bacc(None, target_bir_lowering=False, debug=False)

    Xd = nc.declare_dram_parameter("X", [s_local, D], dt.float32, isOutput=False)
    WeTd = nc.declare_dram_parameter("WeT", [E, D, D], dt.float32, isOutput=False)
    WrTd = nc.declare_dram_parameter("WrT", [D, E], dt.float32, isOutput=False)
    brd = nc.declare_dram_parameter("br", [E, 1], dt.float32, isOutput=False)
    bed = nc.declare_dram_parameter("be", [E, D], dt.float32, isOutput=False)
    WoTd = nc.declare_dram_parameter("WoT", [D, D], dt.float32, isOutput=False)
    bod = nc.declare_dram_parameter("bo", [1, D], dt.float32, isOutput=False)
    outd = nc.declare_dram_parameter("out", [s_local, D], dt.float32, isOutput=True)

    fp32 = dt.float32
    bf16 = dt.bfloat16

    with tile.TileContext(nc) as tc:
        with tc.tile_pool(name="const", bufs=1) as const_p, \
             tc.tile_pool(name="big", bufs=1) as big_p, \
             tc.tile_pool(name="we", bufs=2) as we_p, \
             tc.tile_pool(name="xf", bufs=3) as xf_p, \
             tc.tile_pool(name="xt32", bufs=4) as xt_p, \
             tc.tile_pool(name="sm", bufs=4) as sm_p, \
             tc.tile_pool(name="cmb", bufs=4) as cmb_p, \
             tc.tile_pool(name="oc", bufs=3) as oc_p, \
             tc.tile_pool(name="ps_tr", bufs=2, space="PSUM") as pst_p, \
             tc.tile_pool(name="ps_lg", bufs=2, space="PSUM") as psl_p, \
             tc.tile_pool(name="ps_mm", bufs=2, space="PSUM") as mm_p:

            # ---------- constants ----------
            ID = const_p.tile([P, P], fp32)
            make_identity(nc, ID[:])
            IDb = const_p.tile([P, P], bf16)
            nc.vector.tensor_copy(out=IDb[:], in_=ID[:])
            WrTs = const_p.tile([P, NKT, E], fp32)   # [128, kt, 8]
            nc.sync.dma_start(
                out=WrTs[:], in_=WrTd[:].rearrange("(kt p) e -> p kt e", p=P))
            brS = const_p.tile([E, 1], fp32)
            nc.sync.dma_start(out=brS[:], in_=brd[:])
            be9 = const_p.tile([E + 1, D], fp32)
            nc.sync.dma_start(out=be9[:E, :], in_=bed[:])
            nc.sync.dma_start(out=be9[E:E + 1, :], in_=bod[:])
            WoTb = const_p.tile([P, NKT, D], bf16)   # [128, kt, 1024]
            nc.gpsimd.dma_start(
                out=WoTb[:], in_=WoTd[:].rearrange("(kt p) h -> p kt h", p=P))

            # be9p = [be @ WoT ; bo]  (expert bias folded through output proj)
            beTb = const_p.tile([P, NKT, E + 1], bf16)   # be^T (col E zero), bf16
            nc.vector.memset(beTb[:], 0.0)
            for kt in range(NKT):
                ptb = pst_p.tile([P, E], fp32, space="PSUM", tag="tr")
                nc.tensor.transpose(
                    out=ptb[:], in_=be9[:E, kt * P:(kt + 1) * P],
                    identity=ID[:E, :E])
                nc.vector.tensor_copy(out=beTb[:, kt, :E], in_=ptb[:])
            e9 = const_p.tile([1, E + 1], fp32)
            nc.vector.memset(e9[:], 0.0)
            nc.vector.memset(e9[:, E:], 1.0)
            bo_s = const_p.tile([1, D], fp32)
            nc.sync.dma_start(out=bo_s[:], in_=bod[:])
            ps9 = mm_p.tile([E + 1, D], fp32, space="PSUM", tag="mm")
            for h2 in range(2):
                hsl = slice(h2 * 512, (h2 + 1) * 512)
                for kt in range(NKT):
                    nc.tensor.matmul(
                        out=ps9[:, hsl], lhsT=beTb[:, kt, :],
                        rhs=WoTb[:, kt, hsl], start=(kt == 0), stop=False)
                nc.tensor.matmul(
                    out=ps9[:, hsl], lhsT=e9[:], rhs=bo_s[:, hsl],
                    start=False, stop=True)
            be9p = const_p.tile([E + 1, D], fp32)
            nc.vector.tensor_copy(out=be9p[:], in_=ps9[:])

            # ---------- persistent big tiles ----------
            XTb = big_p.tile([P, NKT, s_local], bf16, tag="xtb")   # X^T bf16
            ACC = big_p.tile([P, nt, D], bf16, tag="acc")          # combined
            Wtop2 = big_p.tile([P, nt * E], fp32, tag="wtop2")     # masked top-2 w

            # ---------- phase 1: load X, transpose, router, top-2 ----------
            for t in range(nt):
                xf = xf_p.tile([P, D], fp32)
                nc.sync.dma_start(out=xf[:], in_=Xd[t * P:(t + 1) * P, :])
                lg = psl_p.tile([E, P], fp32, space="PSUM", tag="lg")
                for kt in range(NKT):
                    pt = pst_p.tile([P, P], fp32, space="PSUM", tag="tr")
                    nc.tensor.transpose(
                        out=pt[:], in_=xf[:, kt * P:(kt + 1) * P], identity=ID[:])
                    xt32 = xt_p.tile([P, P], fp32)
                    nc.vector.tensor_copy(out=xt32[:], in_=pt[:])
                    nc.vector.tensor_copy(
                        out=XTb[:, kt, t * P:(t + 1) * P], in_=pt[:])
                    nc.tensor.matmul(
                        out=lg[:], lhsT=WrTs[:, kt, :], rhs=xt32[:],
                        start=(kt == 0), stop=(kt == NKT - 1))
                # +br on [E, 128] form, then transpose to [128, E]
                lgs = sm_p.tile([E, P], fp32, tag="lgs")
                nc.vector.tensor_scalar(
                    out=lgs[:], in0=lg[:], scalar1=brS[:, 0:1], scalar2=None,
                    op0=mybir.AluOpType.add)
                lp = pst_p.tile([P, E], fp32, space="PSUM", tag="tr")
                nc.tensor.transpose(out=lp[:], in_=lgs[:], identity=ID[:E, :E])
                Ls = sm_p.tile([P, E], fp32, tag="ls")
                nc.vector.tensor_copy(out=Ls[:], in_=lp[:])
                # softmax over 8 experts
                mneg = sm_p.tile([P, 1], fp32, tag="mneg")
                nc.vector.tensor_reduce(
                    out=mneg[:], in_=Ls[:], axis=mybir.AxisListType.X,
                    op=mybir.AluOpType.max, negate=True)
                Eexp = sm_p.tile([P, E], fp32, tag="eexp")
                Zs = sm_p.tile([P, 1], fp32, tag="zs")
                nc.scalar.activation(
                    out=Eexp[:], in_=Ls[:], func=mybir.ActivationFunctionType.Exp,
                    bias=mneg[:, 0:1], scale=1.0, accum_out=Zs[:, 0:1])
                rZ = sm_p.tile([P, 1], fp32, tag="rz")
                nc.vector.reciprocal(out=rZ[:], in_=Zs[:])
                Wsm = sm_p.tile([P, E], fp32, tag="wsm")
                nc.vector.tensor_scalar_mul(Wsm[:], Eexp[:], rZ[:, 0:1])
                # top-2 mask: keep top-2 softmax weights, zero the rest
                Wm8 = sm_p.tile([P, E], fp32, tag="wm8")
                nc.vector.max(out=Wm8[:], in_=Wsm[:])
                mr8 = sm_p.tile([P, E], fp32, tag="mr8")
                nc.vector.tensor_copy(out=mr8[:], in_=Wm8[:])
                nc.vector.memset(mr8[:, TOPK:], -1.0)
                Wz = sm_p.tile([P, E], fp32, tag="wz")
                nc.vector.match_replace(
                    out=Wz[:], in_to_replace=mr8[:], in_values=Wsm[:], imm_value=0.0)
                nc.vector.tensor_sub(
                    out=Wtop2[:, t * E:(t + 1) * E], in0=Wsm[:], in1=Wz[:])

            # ---------- phase 2: dense expert matmuls + weighted combine ----------
            for e in range(E):
                web = we_p.tile([P, NKT, D], bf16)   # WeT[e] cast to bf16
                nc.gpsimd.dma_start(
                    out=web[:], in_=WeTd[e].rearrange("(kt p) h -> p kt h", p=P))
                for t in range(nt):
                    zp = mm_p.tile([P, D], fp32, space="PSUM", tag="mm")
                    for kt in range(NKT):
                        for h2 in range(2):
                            nc.tensor.matmul(
                                out=zp[:, h2 * 512:(h2 + 1) * 512],
                                lhsT=XTb[:, kt, t * P:(t + 1) * P],
                                rhs=web[:, kt, h2 * 512:(h2 + 1) * 512],
                                start=(kt == 0), stop=(kt == NKT - 1))
                    wcol = Wtop2[:, t * E + e:t * E + e + 1]
                    if e == 0:
                        nc.vector.tensor_scalar_mul(ACC[:, t, :], zp[:], wcol)
                    else:
                        tmp = cmb_p.tile([P, D], bf16, tag="cmb")
                        nc.scalar.activation(
                            out=tmp[:], in_=zp[:],
                            func=mybir.ActivationFunctionType.Copy, scale=wcol)
                        nc.vector.tensor_add(
                            out=ACC[:, t, :], in0=ACC[:, t, :], in1=tmp[:])

            # ---------- phase 3: output projection + biases ----------
            for t in range(nt):
                accT = oc_p.tile([P, NKT, P], bf16, tag="acct")
                for kt in range(NKT):
                    ptt = pst_p.tile([P, P], bf16, space="PSUM", tag="tr")
                    nc.tensor.transpose(
                        out=ptt[:], in_=ACC[:, t, kt * P:(kt + 1) * P],
                        identity=IDb[:])
                    nc.vector.tensor_copy(out=accT[:, kt, :], in_=ptt[:])
                # W9 = [Wtop2_t | ones] -> transpose -> [9, 128]
                w9 = oc_p.tile([P, E + 1], fp32, tag="w9")
                nc.vector.tensor_copy(out=w9[:, :E], in_=Wtop2[:, t * E:(t + 1) * E])
                nc.vector.memset(w9[:, E:], 1.0)
                w9tp = pst_p.tile([E + 1, P], fp32, space="PSUM", tag="tr")
                nc.tensor.transpose(out=w9tp[:], in_=w9[:], identity=ID[:])
                w9t = oc_p.tile([E + 1, P], fp32, tag="w9t")
                nc.vector.tensor_copy(out=w9t[:], in_=w9tp[:])

                op = mm_p.tile([P, D], fp32, space="PSUM", tag="mm")
                for h2 in range(2):
                    hsl = slice(h2 * 512, (h2 + 1) * 512)
                    for kt in range(NKT):
                        nc.tensor.matmul(
                            out=op[:, hsl], lhsT=accT[:, kt, :],
                            rhs=WoTb[:, kt, hsl], start=(kt == 0), stop=False)
                    nc.tensor.matmul(
                        out=op[:, hsl], lhsT=w9t[:], rhs=be9p[:, hsl],
                        start=False, stop=True)
                osb = oc_p.tile([P, D], fp32, tag="osb")
                if t % 2 == 0:
                    nc.vector.tensor_copy(out=osb[:], in_=op[:])
                else:
                    nc.scalar.activation(
                        out=osb[:], in_=op[:], func=mybir.ActivationFunctionType.Copy)
                nc.sync.dma_start(out=outd[t * P:(t + 1) * P, :], in_=osb[:])

    nc.compile()
    return nc


_NC_CACHE = {}


def _get_nc(s_local=S):
    if s_local not in _NC_CACHE:
        _NC_CACHE[s_local] = build_kernel(s_local)
    return _NC_CACHE[s_local]


def make_in_maps(X, We, be, Wr, br, Wo, bo):
    WeT = np.ascontiguousarray(np.transpose(np.asarray(We), (0, 2, 1)), np.float32)
    WrT = np.ascontiguousarray(np.asarray(Wr).T, np.float32)
    WoT = np.ascontiguousarray(np.asarray(Wo).T, np.float32)
    brC = np.ascontiguousarray(np.asarray(br, np.float32).reshape(E, 1))
    beC = np.ascontiguousarray(np.asarray(be), np.float32)
    boC = np.ascontiguousarray(np.asarray(bo, np.float32).reshape(1, D))
    Xc = np.asarray(X, np.float32)
    return [
        {"X": np.ascontiguousarray(Xc[c]), "WeT": WeT, "WrT": WrT, "br": brC,
         "be": beC, "WoT": WoT, "bo": boC}
        for c in range(B)
    ]


def kernel(X, We, be, Wr, br, Wo, bo):
    from concourse.bass_utils import run_bass_kernel_spmd
    nc = _get_nc()
    in_maps = make_in_maps(X, We, be, Wr, br, Wo, bo)
    res = run_bass_kernel_spmd(nc, in_maps, list(range(B)))
    out = np.stack([res.results[c]["out"] for c in range(B)], axis=0)
    return out.astype(np.float32)


# revision 7
# speedup vs baseline: 1.2995x; 1.2995x over previous
# MoE top-2 routing kernel for Trainium2, 8 NeuronCores, data-parallel over batch.
#
# Dense-expert formulation: every expert matmul is computed for every token tile
# (E=8 is small), combined with masked top-2 softmax weights, then projected.
# No dispatch/scatter/gather and no DRAM scratch round trips.
#
# Host->device traffic is minimized (it dominates wall time on this stack):
#   - expert weights ship expert-sharded in bf16 (each core carries only its
#     own expert slice + a column slice of Wo) and are all-gathered on-device
#   - the output ships back as bf16
# Self-contained: hardcodes shapes B=8, S=2048, D=1024, E=8, TOP_K=2.
import numpy as np
import ml_dtypes

B, S, D, E = 8, 2048, 1024, 8
TOPK = 2
P = 128
NKT = D // P             # 8 contraction tiles
NT = S // P              # 16 token tiles per core
WSH = D + P              # per-core weight shard columns: [WeT_c | WoT col-slice]


def build_kernel(s_local=S, n_cores=B):
    import concourse.bacc as bacc
    import concourse.tile as tile
    import concourse.mybir as mybir
    from concourse.masks import make_identity

    dt = mybir.dt
    nt = s_local // P

    nc = bacc.Bacc(None, target_bir_lowering=False, debug=False,
                   num_devices=n_cores)

    Xd = nc.declare_dram_parameter("X", [s_local, D], dt.float32, isOutput=False)
    Wshd = nc.declare_dram_parameter("Wsh", [D, WSH], dt.bfloat16, isOutput=False)
    WrTd = nc.declare_dram_parameter("WrT", [D, E], dt.float32, isOutput=False)
    brd = nc.declare_dram_parameter("br", [E, 1], dt.float32, isOutput=False)
    bed = nc.declare_dram_parameter("be", [E, D], dt.float32, isOutput=False)
    bod = nc.declare_dram_parameter("bo", [1, D], dt.float32, isOutput=False)
    outd = nc.declare_dram_parameter("out", [s_local, D], dt.bfloat16,
                                     isOutput=True)

    fp32 = dt.float32
    bf16 = dt.bfloat16

    # internal DRAM for the weight all-gather
    WshIn = nc.dram_tensor("wsh_in", [D, WSH], bf16)
    WG = nc.dram_tensor("wsh_all", [n_cores, D, WSH], bf16, addr_space="Shared")

    with tile.TileContext(nc) as tc:
        with tc.tile_pool(name="const", bufs=1) as const_p, \
             tc.tile_pool(name="big", bufs=1) as big_p, \
             tc.tile_pool(name="we", bufs=2) as we_p, \
             tc.tile_pool(name="xf", bufs=3) as xf_p, \
             tc.tile_pool(name="xt32", bufs=4) as xt_p, \
             tc.tile_pool(name="sm", bufs=4) as sm_p, \
             tc.tile_pool(name="cmb", bufs=4) as cmb_p, \
             tc.tile_pool(name="oc", bufs=3) as oc_p, \
             tc.tile_pool(name="ps_tr", bufs=2, space="PSUM") as pst_p, \
             tc.tile_pool(name="ps_lg", bufs=2, space="PSUM") as psl_p, \
             tc.tile_pool(name="ps_mm", bufs=2, space="PSUM") as mm_p:

            # ---------- weight all-gather (overlaps router phase below) ----------
            nc.sync.dma_start(out=WshIn[:], in_=Wshd[:])
            nc.gpsimd.collective_compute(
                "AllGather", mybir.AluOpType.bypass,
                replica_groups=[[i for i in range(n_cores)]],
                ins=[WshIn[:]], outs=[WG[:]])

            # ---------- constants ----------
            ID = const_p.tile([P, P], fp32)
            make_identity(nc, ID[:])
            IDb = const_p.tile([P, P], bf16)
            nc.vector.tensor_copy(out=IDb[:], in_=ID[:])
            WrTs = const_p.tile([P, NKT, E], fp32)   # [128, kt, 8]
            nc.sync.dma_start(
                out=WrTs[:], in_=WrTd[:].rearrange("(kt p) e -> p kt e", p=P))
            brS = const_p.tile([E, 1], fp32)
            nc.sync.dma_start(out=brS[:], in_=brd[:])
            be9 = const_p.tile([E + 1, D], fp32)
            nc.sync.dma_start(out=be9[:E, :], in_=bed[:])
            nc.sync.dma_start(out=be9[E:E + 1, :], in_=bod[:])
            # WoT assembled from the gathered shards: [128, kt, 1024] bf16
            WoTb = const_p.tile([P, NKT, D], bf16)
            for c in range(n_cores):
                nc.sync.dma_start(
                    out=WoTb[:, :, c * P:(c + 1) * P],
                    in_=WG[c][:, D:].rearrange("(kt p) h -> p kt h", p=P))

            # be9p = [be @ WoT ; bo]  (expert bias folded through output proj)
            beTb = const_p.tile([P, NKT, E + 1], bf16)   # be^T (col E zero), bf16
            nc.vector.memset(beTb[:], 0.0)
            for kt in range(NKT):
                ptb = pst_p.tile([P, E], fp32, space="PSUM", tag="tr")
                nc.tensor.transpose(
                    out=ptb[:], in_=be9[:E, kt * P:(kt + 1) * P],
                    identity=ID[:E, :E])
                nc.vector.tensor_copy(out=beTb[:, kt, :E], in_=ptb[:])
            e9 = const_p.tile([1, E + 1], fp32)
            nc.vector.memset(e9[:], 0.0)
            nc.vector.memset(e9[:, E:], 1.0)
            bo_s = const_p.tile([1, D], fp32)
            nc.sync.dma_start(out=bo_s[:], in_=bod[:])
            ps9 = mm_p.tile([E + 1, D], fp32, space="PSUM", tag="mm")
            for h2 in range(2):
                hsl = slice(h2 * 512, (h2 + 1) * 512)
                for kt in range(NKT):
                    nc.tensor.matmul(
                        out=ps9[:, hsl], lhsT=beTb[:, kt, :],
                        rhs=WoTb[:, kt, hsl], start=(kt == 0), stop=False)
                nc.tensor.matmul(
                    out=ps9[:, hsl], lhsT=e9[:], rhs=bo_s[:, hsl],
                    start=False, stop=True)
            be9p = const_p.tile([E + 1, D], fp32)
            nc.vector.tensor_copy(out=be9p[:], in_=ps9[:])

            # ---------- persistent big tiles ----------
            XTb = big_p.tile([P, NKT, s_local], bf16, tag="xtb")   # X^T bf16
            ACC = big_p.tile([P, nt, D], bf16, tag="acc")          # combined
            Wtop2 = big_p.tile([P, nt * E], fp32, tag="wtop2")     # masked top-2 w

            # ---------- phase 1: load X, transpose, router, top-2 ----------
            for t in range(nt):
                xf = xf_p.tile([P, D], fp32)
                nc.sync.dma_start(out=xf[:], in_=Xd[t * P:(t + 1) * P, :])
                lg = psl_p.tile([E, P], fp32, space="PSUM", tag="lg")
                for kt in range(NKT):
                    pt = pst_p.tile([P, P], fp32, space="PSUM", tag="tr")
                    nc.tensor.transpose(
                        out=pt[:], in_=xf[:, kt * P:(kt + 1) * P], identity=ID[:])
                    xt32 = xt_p.tile([P, P], fp32)
                    nc.vector.tensor_copy(out=xt32[:], in_=pt[:])
                    nc.vector.tensor_copy(
                        out=XTb[:, kt, t * P:(t + 1) * P], in_=pt[:])
                    nc.tensor.matmul(
                        out=lg[:], lhsT=WrTs[:, kt, :], rhs=xt32[:],
                        start=(kt == 0), stop=(kt == NKT - 1))
                # +br on [E, 128] form, then transpose to [128, E]
                lgs = sm_p.tile([E, P], fp32, tag="lgs")
                nc.vector.tensor_scalar(
                    out=lgs[:], in0=lg[:], scalar1=brS[:, 0:1], scalar2=None,
                    op0=mybir.AluOpType.add)
                lp = pst_p.tile([P, E], fp32, space="PSUM", tag="tr")
                nc.tensor.transpose(out=lp[:], in_=lgs[:], identity=ID[:E, :E])
                Ls = sm_p.tile([P, E], fp32, tag="ls")
                nc.vector.tensor_copy(out=Ls[:], in_=lp[:])
                # softmax over 8 experts
                mneg = sm_p.tile([P, 1], fp32, tag="mneg")
                nc.vector.tensor_reduce(
                    out=mneg[:], in_=Ls[:], axis=mybir.AxisListType.X,
                    op=mybir.AluOpType.max, negate=True)
                Eexp = sm_p.tile([P, E], fp32, tag="eexp")
                Zs = sm_p.tile([P, 1], fp32, tag="zs")
                nc.scalar.activation(
                    out=Eexp[:], in_=Ls[:], func=mybir.ActivationFunctionType.Exp,
                    bias=mneg[:, 0:1], scale=1.0, accum_out=Zs[:, 0:1])
                rZ = sm_p.tile([P, 1], fp32, tag="rz")
                nc.vector.reciprocal(out=rZ[:], in_=Zs[:])
                Wsm = sm_p.tile([P, E], fp32, tag="wsm")
                nc.vector.tensor_scalar_mul(Wsm[:], Eexp[:], rZ[:, 0:1])
                # top-2 mask: keep top-2 softmax weights, zero the rest
                Wm8 = sm_p.tile([P, E], fp32, tag="wm8")
                nc.vector.max(out=Wm8[:], in_=Wsm[:])
                mr8 = sm_p.tile([P, E], fp32, tag="mr8")
                nc.vector.tensor_copy(out=mr8[:], in_=Wm8[:])
                nc.vector.memset(mr8[:, TOPK:], -1.0)
                Wz = sm_p.tile([P, E], fp32, tag="wz")
                nc.vector.match_replace(
                    out=Wz[:], in_to_replace=mr8[:], in_values=Wsm[:], imm_value=0.0)
                nc.vector.tensor_sub(
                    out=Wtop2[:, t * E:(t + 1) * E], in0=Wsm[:], in1=Wz[:])

            # ---------- phase 2: dense expert matmuls + weighted combine ----------
            for e in range(E):
                web = we_p.tile([P, NKT, D], bf16)   # WeT[e] from the all-gather
                nc.sync.dma_start(
                    out=web[:],
                    in_=WG[e][:, :D].rearrange("(kt p) h -> p kt h", p=P))
                for t in range(nt):
                    zp = mm_p.tile([P, D], fp32, space="PSUM", tag="mm")
                    for kt in range(NKT):
                        for h2 in range(2):
                            nc.tensor.matmul(
                                out=zp[:, h2 * 512:(h2 + 1) * 512],
                                lhsT=XTb[:, kt, t * P:(t + 1) * P],
                                rhs=web[:, kt, h2 * 512:(h2 + 1) * 512],
                                start=(kt == 0), stop=(kt == NKT - 1))
                    wcol = Wtop2[:, t * E + e:t * E + e + 1]
                    if e == 0:
                        nc.vector.tensor_scalar_mul(ACC[:, t, :], zp[:], wcol)
                    else:
                        tmp = cmb_p.tile([P, D], bf16, tag="cmb")
                        nc.scalar.activation(
                            out=tmp[:], in_=zp[:],
                            func=mybir.ActivationFunctionType.Copy, scale=wcol)
                        nc.vector.tensor_add(
                            out=ACC[:, t, :], in0=ACC[:, t, :], in1=tmp[:])

            # ---------- phase 3: output projection + biases ----------
            for t in range(nt):
                accT = oc_p.tile([P, NKT, P], bf16, tag="acct")
                for kt in range(NKT):
                    ptt = pst_p.tile([P, P], bf16, space="PSUM", tag="tr")
                    nc.tensor.transpose(
                        out=ptt[:], in_=ACC[:, t, kt * P:(kt + 1) * P],
                        identity=IDb[:])
                    nc.vector.tensor_copy(out=accT[:, kt, :], in_=ptt[:])
                # W9 = [Wtop2_t | ones] -> transpose -> [9, 128]
                w9 = oc_p.tile([P, E + 1], fp32, tag="w9")
                nc.vector.tensor_copy(out=w9[:, :E], in_=Wtop2[:, t * E:(t + 1) * E])
                nc.vector.memset(w9[:, E:], 1.0)
                w9tp = pst_p.tile([E + 1, P], fp32, space="PSUM", tag="tr")
                nc.tensor.transpose(out=w9tp[:], in_=w9[:], identity=ID[:])
                w9t = oc_p.tile([E + 1, P], fp32, tag="w9t")
                nc.vector.tensor_copy(out=w9t[:], in_=w9tp[:])

                op = mm_p.tile([P, D], fp32, space="PSUM", tag="mm")
                for h2 in range(2):
                    hsl = slice(h2 * 512, (h2 + 1) * 512)
                    for kt in range(NKT):
                        nc.tensor.matmul(
                            out=op[:, hsl], lhsT=accT[:, kt, :],
                            rhs=WoTb[:, kt, hsl], start=(kt == 0), stop=False)
                    nc.tensor.matmul(
                        out=op[:, hsl], lhsT=w9t[:], rhs=be9p[:, hsl],
                        start=False, stop=True)
                osb = oc_p.tile([P, D], bf16, tag="osb")
                if t % 2 == 0:
                    nc.vector.tensor_copy(out=osb[:], in_=op[:])
                else:
                    nc.scalar.activation(
                        out=osb[:], in_=op[:], func=mybir.ActivationFunctionType.Copy)
                nc.sync.dma_start(out=outd[t * P:(t + 1) * P, :], in_=osb[:])

    nc.compile()
    return nc


_NC_CACHE = {}


def _get_nc(s_local=S):
    if s_local not in _NC_CACHE:
        _NC_CACHE[s_local] = build_kernel(s_local)
    return _NC_CACHE[s_local]


def make_in_maps(X, We, be, Wr, br, Wo, bo):
    bf = ml_dtypes.bfloat16
    We = np.asarray(We, np.float32)
    WoT = np.asarray(Wo, np.float32).T            # [d, h]
    WrT = np.ascontiguousarray(np.asarray(Wr).T, np.float32)
    brC = np.ascontiguousarray(np.asarray(br, np.float32).reshape(E, 1))
    beC = np.ascontiguousarray(np.asarray(be), np.float32)
    boC = np.ascontiguousarray(np.asarray(bo, np.float32).reshape(1, D))
    Xc = np.asarray(X, np.float32)
    maps = []
    for c in range(B):
        wsh = np.empty((D, WSH), bf)
        wsh[:, :D] = We[c].T.astype(bf)           # WeT_c [d, h]
        wsh[:, D:] = WoT[:, c * P:(c + 1) * P].astype(bf)
        maps.append({"X": np.ascontiguousarray(Xc[c]), "Wsh": wsh, "WrT": WrT,
                     "br": brC, "be": beC, "bo": boC})
    return maps


def kernel(X, We, be, Wr, br, Wo, bo):
    from concourse.bass_utils import run_bass_kernel_spmd
    nc = _get_nc()
    in_maps = make_in_maps(X, We, be, Wr, br, Wo, bo)
    res = run_bass_kernel_spmd(nc, in_maps, list(range(B)))
    out = np.stack([res.results[c]["out"] for c in range(B)], axis=0)
    return out.astype(np.float32)


# revision 13
# speedup vs baseline: 1.3431x; 1.0336x over previous
# MoE top-2 routing kernel for Trainium2, 8 NeuronCores, data-parallel over batch.
#
# Dense-expert formulation: every expert matmul is computed for every token tile
# (E=8 is small), combined with masked top-2 softmax weights, then projected.
# No dispatch/scatter/gather and no DRAM scratch round trips.
#
# Host->device traffic is minimized (it dominates wall time on this stack):
#   - expert weights ship expert-sharded in bf16 (each core carries only its
#     own expert slice + a column slice of Wo) and are all-gathered on-device
#   - the output ships back as bf16
# Self-contained: hardcodes shapes B=8, S=2048, D=1024, E=8, TOP_K=2.
import numpy as np
import ml_dtypes

B, S, D, E = 8, 2048, 1024, 8
TOPK = 2
P = 128
NKT = D // P             # 8 contraction tiles
NT = S // P              # 16 token tiles per core
WSH = D + P              # per-core weight shard columns: [WeT_c | WoT col-slice]
XTRA = E + 1 + E + 1     # extra X rows carrying WrT cols, br, be, bo


def build_kernel(s_local=S, n_cores=B):
    import concourse.bacc as bacc
    import concourse.tile as tile
    import concourse.mybir as mybir
    from concourse.masks import make_identity

    dt = mybir.dt
    nt = s_local // P

    nc = bacc.Bacc(None, target_bir_lowering=False, debug=False,
                   num_devices=n_cores)

    # X is extended with XTRA rows carrying the small fp32 tensors
    # (WrT columns, br, be, bo) to minimize the per-call buffer count.
    Xd = nc.declare_dram_parameter("X", [s_local + XTRA, D], dt.float32,
                                   isOutput=False)
    Wshd = nc.declare_dram_parameter("Wsh", [D, WSH], dt.bfloat16, isOutput=False)
    outd = nc.declare_dram_parameter("out", [s_local, D], dt.bfloat16,
                                     isOutput=True)
    # row offsets of the small tensors inside Xd
    R_WRT = s_local            # rows R_WRT..R_WRT+8: WrT column e (length D)
    R_BR = s_local + E         # one row: br in first E cols
    R_BE = s_local + E + 1     # 8 rows: be[e]
    R_BO = s_local + 2 * E + 1  # one row: bo

    fp32 = dt.float32
    bf16 = dt.bfloat16

    # internal DRAM for the weight all-gather
    WshIn = nc.dram_tensor("wsh_in", [D, WSH], bf16)
    WG = nc.dram_tensor("wsh_all", [n_cores, D, WSH], bf16, addr_space="Shared")

    with tile.TileContext(nc) as tc:
        with tc.tile_pool(name="const", bufs=1) as const_p, \
             tc.tile_pool(name="big", bufs=1) as big_p, \
             tc.tile_pool(name="we", bufs=2) as we_p, \
             tc.tile_pool(name="xf", bufs=3) as xf_p, \
             tc.tile_pool(name="xt32", bufs=4) as xt_p, \
             tc.tile_pool(name="sm", bufs=4) as sm_p, \
             tc.tile_pool(name="cmb", bufs=4) as cmb_p, \
             tc.tile_pool(name="oc", bufs=3) as oc_p, \
             tc.tile_pool(name="ps_tr", bufs=2, space="PSUM") as pst_p, \
             tc.tile_pool(name="ps_lg", bufs=2, space="PSUM") as psl_p, \
             tc.tile_pool(name="ps_mm", bufs=2, space="PSUM") as mm_p:

            # ---------- weight all-gather (overlaps router phase below) ----------
            nc.sync.dma_start(out=WshIn[:], in_=Wshd[:])
            nc.gpsimd.collective_compute(
                "AllGather", mybir.AluOpType.bypass,
                replica_groups=[[i for i in range(n_cores)]],
                ins=[WshIn[:]], outs=[WG[:]])

            # ---------- constants ----------
            ID = const_p.tile([P, P], fp32)
            make_identity(nc, ID[:])
            IDb = const_p.tile([P, P], bf16)
            nc.vector.tensor_copy(out=IDb[:], in_=ID[:])
            WrTs = const_p.tile([P, NKT, E], fp32)   # [128, kt, 8]
            for e in range(E):
                nc.sync.dma_start(
                    out=WrTs[:, :, e:e + 1],
                    in_=Xd[R_WRT + e:R_WRT + e + 1, :].rearrange(
                        "o (kt p) -> p kt o", p=P))
            brS = const_p.tile([E, 1], fp32)
            nc.sync.dma_start(
                out=brS[:], in_=Xd[R_BR:R_BR + 1, :E].rearrange("o e -> e o"))
            be9 = const_p.tile([E + 1, D], fp32)
            nc.sync.dma_start(out=be9[:E, :], in_=Xd[R_BE:R_BE + E, :])
            nc.sync.dma_start(out=be9[E:E + 1, :], in_=Xd[R_BO:R_BO + 1, :])
            # WoT assembled from the gathered shards: [128, kt, 1024] bf16
            WoTb = const_p.tile([P, NKT, D], bf16)
            for c in range(n_cores):
                nc.sync.dma_start(
                    out=WoTb[:, :, c * P:(c + 1) * P],
                    in_=WG[c][:, D:].rearrange("(kt p) h -> p kt h", p=P))

            # be9p = [be @ WoT ; bo]  (expert bias folded through output proj)
            beTb = const_p.tile([P, NKT, E + 1], bf16)   # be^T (col E zero), bf16
            nc.vector.memset(beTb[:], 0.0)
            for kt in range(NKT):
                ptb = pst_p.tile([P, E], fp32, space="PSUM", tag="tr")
                nc.tensor.transpose(
                    out=ptb[:], in_=be9[:E, kt * P:(kt + 1) * P],
                    identity=ID[:E, :E])
                nc.vector.tensor_copy(out=beTb[:, kt, :E], in_=ptb[:])
            e9 = const_p.tile([1, E + 1], fp32)
            nc.vector.memset(e9[:], 0.0)
            nc.vector.memset(e9[:, E:], 1.0)
            bo_s = const_p.tile([1, D], fp32)
            nc.sync.dma_start(out=bo_s[:], in_=Xd[R_BO:R_BO + 1, :])
            ps9 = mm_p.tile([E + 1, D], fp32, space="PSUM", tag="mm")
            for h2 in range(2):
                hsl = slice(h2 * 512, (h2 + 1) * 512)
                for kt in range(NKT):
                    nc.tensor.matmul(
                        out=ps9[:, hsl], lhsT=beTb[:, kt, :],
                        rhs=WoTb[:, kt, hsl], start=(kt == 0), stop=False)
                nc.tensor.matmul(
                    out=ps9[:, hsl], lhsT=e9[:], rhs=bo_s[:, hsl],
                    start=False, stop=True)
            be9p = const_p.tile([E + 1, D], fp32)
            nc.vector.tensor_copy(out=be9p[:], in_=ps9[:])

            # ---------- persistent big tiles ----------
            XTb = big_p.tile([P, NKT, s_local], bf16, tag="xtb")   # X^T bf16
            ACC = big_p.tile([P, nt, D], bf16, tag="acc")          # combined
            Wtop2 = big_p.tile([P, nt * E], fp32, tag="wtop2")     # masked top-2 w

            # ---------- phase 1: load X, transpose, router, top-2 ----------
            for t in range(nt):
                xf = xf_p.tile([P, D], fp32)
                nc.sync.dma_start(out=xf[:], in_=Xd[t * P:(t + 1) * P, :])
                lg = psl_p.tile([E, P], fp32, space="PSUM", tag="lg")
                for kt in range(NKT):
                    pt = pst_p.tile([P, P], fp32, space="PSUM", tag="tr")
                    nc.tensor.transpose(
                        out=pt[:], in_=xf[:, kt * P:(kt + 1) * P], identity=ID[:])
                    xt32 = xt_p.tile([P, P], fp32)
                    nc.vector.tensor_copy(out=xt32[:], in_=pt[:])
                    nc.vector.tensor_copy(
                        out=XTb[:, kt, t * P:(t + 1) * P], in_=pt[:])
                    nc.tensor.matmul(
                        out=lg[:], lhsT=WrTs[:, kt, :], rhs=xt32[:],
                        start=(kt == 0), stop=(kt == NKT - 1))
                # +br on [E, 128] form, then transpose to [128, E]
                lgs = sm_p.tile([E, P], fp32, tag="lgs")
                nc.vector.tensor_scalar(
                    out=lgs[:], in0=lg[:], scalar1=brS[:, 0:1], scalar2=None,
                    op0=mybir.AluOpType.add)
                lp = pst_p.tile([P, E], fp32, space="PSUM", tag="tr")
                nc.tensor.transpose(out=lp[:], in_=lgs[:], identity=ID[:E, :E])
                Ls = sm_p.tile([P, E], fp32, tag="ls")
                nc.vector.tensor_copy(out=Ls[:], in_=lp[:])
                # softmax over 8 experts
                mneg = sm_p.tile([P, 1], fp32, tag="mneg")
                nc.vector.tensor_reduce(
                    out=mneg[:], in_=Ls[:], axis=mybir.AxisListType.X,
                    op=mybir.AluOpType.max, negate=True)
                Eexp = sm_p.tile([P, E], fp32, tag="eexp")
                Zs = sm_p.tile([P, 1], fp32, tag="zs")
                nc.scalar.activation(
                    out=Eexp[:], in_=Ls[:], func=mybir.ActivationFunctionType.Exp,
                    bias=mneg[:, 0:1], scale=1.0, accum_out=Zs[:, 0:1])
                rZ = sm_p.tile([P, 1], fp32, tag="rz")
                nc.vector.reciprocal(out=rZ[:], in_=Zs[:])
                Wsm = sm_p.tile([P, E], fp32, tag="wsm")
                nc.vector.tensor_scalar_mul(Wsm[:], Eexp[:], rZ[:, 0:1])
                # top-2 mask: keep top-2 softmax weights, zero the rest
                Wm8 = sm_p.tile([P, E], fp32, tag="wm8")
                nc.vector.max(out=Wm8[:], in_=Wsm[:])
                mr8 = sm_p.tile([P, E], fp32, tag="mr8")
                nc.vector.tensor_copy(out=mr8[:], in_=Wm8[:])
                nc.vector.memset(mr8[:, TOPK:], -1.0)
                Wz = sm_p.tile([P, E], fp32, tag="wz")
                nc.vector.match_replace(
                    out=Wz[:], in_to_replace=mr8[:], in_values=Wsm[:], imm_value=0.0)
                nc.vector.tensor_sub(
                    out=Wtop2[:, t * E:(t + 1) * E], in0=Wsm[:], in1=Wz[:])

            # ---------- phase 2: dense expert matmuls + weighted combine ----------
            for e in range(E):
                web = we_p.tile([P, NKT, D], bf16)   # WeT[e] from the all-gather
                nc.sync.dma_start(
                    out=web[:],
                    in_=WG[e][:, :D].rearrange("(kt p) h -> p kt h", p=P))
                for t in range(nt):
                    zp = mm_p.tile([P, D], fp32, space="PSUM", tag="mm")
                    for kt in range(NKT):
                        for h2 in range(2):
                            nc.tensor.matmul(
                                out=zp[:, h2 * 512:(h2 + 1) * 512],
                                lhsT=XTb[:, kt, t * P:(t + 1) * P],
                                rhs=web[:, kt, h2 * 512:(h2 + 1) * 512],
                                start=(kt == 0), stop=(kt == NKT - 1))
                    wcol = Wtop2[:, t * E + e:t * E + e + 1]
                    if e == 0:
                        nc.vector.tensor_scalar_mul(ACC[:, t, :], zp[:], wcol)
                    else:
                        tmp = cmb_p.tile([P, D], bf16, tag="cmb")
                        nc.scalar.activation(
                            out=tmp[:], in_=zp[:],
                            func=mybir.ActivationFunctionType.Copy, scale=wcol)
                        nc.vector.tensor_add(
                            out=ACC[:, t, :], in0=ACC[:, t, :], in1=tmp[:])

            # ---------- phase 3: output projection + biases ----------
            for t in range(nt):
                accT = oc_p.tile([P, NKT, P], bf16, tag="acct")
                for kt in range(NKT):
                    ptt = pst_p.tile([P, P], bf16, space="PSUM", tag="tr")
                    nc.tensor.transpose(
                        out=ptt[:], in_=ACC[:, t, kt * P:(kt + 1) * P],
                        identity=IDb[:])
                    nc.vector.tensor_copy(out=accT[:, kt, :], in_=ptt[:])
                # W9 = [Wtop2_t | ones] -> transpose -> [9, 128]
                w9 = oc_p.tile([P, E + 1], fp32, tag="w9")
                nc.vector.tensor_copy(out=w9[:, :E], in_=Wtop2[:, t * E:(t + 1) * E])
                nc.vector.memset(w9[:, E:], 1.0)
                w9tp = pst_p.tile([E + 1, P], fp32, space="PSUM", tag="tr")
                nc.tensor.transpose(out=w9tp[:], in_=w9[:], identity=ID[:])
                w9t = oc_p.tile([E + 1, P], fp32, tag="w9t")
                nc.vector.tensor_copy(out=w9t[:], in_=w9tp[:])

                op = mm_p.tile([P, D], fp32, space="PSUM", tag="mm")
                for h2 in range(2):
                    hsl = slice(h2 * 512, (h2 + 1) * 512)
                    for kt in range(NKT):
                        nc.tensor.matmul(
                            out=op[:, hsl], lhsT=accT[:, kt, :],
                            rhs=WoTb[:, kt, hsl], start=(kt == 0), stop=False)
                    nc.tensor.matmul(
                        out=op[:, hsl], lhsT=w9t[:], rhs=be9p[:, hsl],
                        start=False, stop=True)
                osb = oc_p.tile([P, D], bf16, tag="osb")
                if t % 2 == 0:
                    nc.vector.tensor_copy(out=osb[:], in_=op[:])
                else:
                    nc.scalar.activation(
                        out=osb[:], in_=op[:], func=mybir.ActivationFunctionType.Copy)
                nc.sync.dma_start(out=outd[t * P:(t + 1) * P, :], in_=osb[:])

    nc.compile()
    return nc


_NC_CACHE = {}


def _get_nc(s_local=S):
    if s_local not in _NC_CACHE:
        _NC_CACHE[s_local] = build_kernel(s_local)
    return _NC_CACHE[s_local]


def make_in_maps(X, We, be, Wr, br, Wo, bo):
    bf = ml_dtypes.bfloat16
    We = np.asarray(We, np.float32)
    WoT = np.asarray(Wo, np.float32).T            # [d, h]
    Xc = np.asarray(X, np.float32)
    s_local = Xc.shape[1]
    # extra rows appended to each core's X: WrT columns, br, be, bo
    extra = np.zeros((XTRA, D), np.float32)
    extra[:E, :] = np.asarray(Wr, np.float32)     # row e = Wr[e] = WrT[:, e]
    extra[E, :E] = np.asarray(br, np.float32).reshape(E)
    extra[E + 1:2 * E + 1, :] = np.asarray(be, np.float32)
    extra[2 * E + 1, :] = np.asarray(bo, np.float32).reshape(D)
    maps = []
    for c in range(B):
        xe = np.empty((s_local + XTRA, D), np.float32)
        xe[:s_local] = Xc[c]
        xe[s_local:] = extra
        wsh = np.empty((D, WSH), bf)
        wsh[:, :D] = We[c].T.astype(bf)           # WeT_c [d, h]
        wsh[:, D:] = WoT[:, c * P:(c + 1) * P].astype(bf)
        maps.append({"X": xe, "Wsh": wsh})
    return maps


def kernel(X, We, be, Wr, br, Wo, bo):
    from concourse.bass_utils import run_bass_kernel_spmd
    nc = _get_nc()
    in_maps = make_in_maps(X, We, be, Wr, br, Wo, bo)
    res = run_bass_kernel_spmd(nc, in_maps, list(range(B)))
    out = np.stack([res.results[c]["out"] for c in range(B)], axis=0)
    return out.astype(np.float32)
